# revision 1
# baseline (speedup 1.0000x reference)
"""Trainium2 Bass kernel for nn_AttentionHead_86715389706346.

Mathematical background
-----------------------
The reference module computes, per batch b:
    q = x @ Wq ; k = x @ Wk ; v = x @ Wv            (x: [T, C])
    attn = (q @ k.T) / sqrt(d)                       [T, T]
    attn = attn @ mask          (mask is all ones)
    p    = softmax(attn, axis=0)  (over the query axis)
    out  = p @ v

Because mask is the all-ones matrix, (attn @ mask)[q, t] = sum_k attn[q, k]
is independent of t.  The softmax over the *query* axis of a column-constant
matrix is also column-constant, so p[q, t] = softmax_q(s)[q] where

    s[q] = q[q, :] . ksum / sqrt(d),   ksum = sum_t k[t, :] = (sum_t x[t, :]) @ Wk

and the output collapses to a rank-1 outer product:

    out[q, d] = softmax(s)[q] * vsum[d],   vsum = (sum_t x[t, :]) @ Wv

This identity is exact (it is just a reassociation of the same floating point
sums), so the kernel computes it directly.  s is computed as
    s = x @ wq,  wq = (Wq @ ksum) / sqrt(d)
which contracts x once with a single vector.

Distribution: data-parallel over the batch dimension.  B == 8 == number of
NeuronCores, so core i processes batch i entirely locally; no collectives.
"""

import numpy as np

T = 2048
IN_C = 1024
D = 128
P = 128
NT = T // P      # 16 token tiles
NC = IN_C // P   # 8 channel chunks
B = 8
ALPHA = float(1.0 / np.sqrt(128.0))

_NC_CACHE = {}


def build_bass():
    import concourse.bass as bass
    import concourse.bacc as bacc
    import concourse.mybir as mybir
    import concourse.tile as tile
    from concourse.masks import make_identity
    from concourse.bass import ts

    f32 = mybir.dt.float32
    AF = mybir.ActivationFunctionType

    nc = bacc.Bacc()
    x_ext = nc.declare_dram_parameter("x", [T, IN_C], f32, isOutput=False)
    wq_ext = nc.declare_dram_parameter("Wq", [IN_C, D], f32, isOutput=False)
    wk_ext = nc.declare_dram_parameter("Wk", [IN_C, D], f32, isOutput=False)
    wv_ext = nc.declare_dram_parameter("Wv", [IN_C, D], f32, isOutput=False)
    out_ext = nc.declare_dram_parameter("out", [T, D], f32, isOutput=True)

    x_tiles = x_ext.rearrange("(i p) c -> i p c", p=P)        # [16, 128, 1024]
    out_view = out_ext.rearrange("(i p) d -> p i d", p=P)     # [128, 16, 128]

    with tile.TileContext(nc) as tc:
        with (
            tc.tile_pool(name="const", bufs=1) as cpool,
            tc.tile_pool(name="xin", bufs=3) as xin,
            tc.tile_pool(name="big", bufs=1) as big,
            tc.tile_pool(name="small", bufs=1) as small,
            tc.tile_pool(name="pxt", bufs=3, space="PSUM") as pxt,
            tc.tile_pool(name="pacc", bufs=2, space="PSUM") as pacc,
            tc.tile_pool(name="pbig", bufs=1, space="PSUM") as pbig,
        ):
            # ---- constants ----
            ident = cpool.tile([P, P], f32)
            make_identity(nc, ident)
            ones_col = cpool.tile([P, 1], f32)
            nc.vector.memset(ones_col, 1.0)
            ones_row = cpool.tile([1, P], f32)
            nc.vector.memset(ones_row, 1.0)

            # ---- weights: load as [c_in, chunk, d] ----
            wq_sb = cpool.tile([P, NC, D], f32)
            nc.sync.dma_start(out=wq_sb, in_=wq_ext.rearrange("(j c) d -> c j d", c=P))
            wk_sb = cpool.tile([P, NC, D], f32)
            nc.sync.dma_start(out=wk_sb, in_=wk_ext.rearrange("(j c) d -> c j d", c=P))
            wv_sb = cpool.tile([P, NC, D], f32)
            nc.sync.dma_start(out=wv_sb, in_=wv_ext.rearrange("(j c) d -> c j d", c=P))

            # WqT (scaled by 1/sqrt(d)): [d, chunk, c]
            wqT = cpool.tile([P, NC, P], f32)
            for j in range(NC):
                pt = pacc.tile([P, P], f32, tag="sm")
                nc.tensor.transpose(pt, wq_sb[:, j, :], ident)
                nc.scalar.activation(out=wqT[:, j, :], in_=pt, func=AF.Copy,
                                     scale=ALPHA)

            # Preload the exp table while DMA streams.
            dummy = small.tile([P, 1], f32, tag="dummy")
            nc.scalar.activation(out=dummy, in_=ones_col, func=AF.Exp)

            # ---- stream x: DMA in, transpose, accumulate ----
            xT = big.tile([P, NT, NC, P], f32, tag="xT")   # [c_in, i, j, t_in]
            xacc = small.tile([P, IN_C], f32, tag="xacc")  # [t_in, c]
            for i in range(NT):
                xt = xin.tile([P, IN_C], f32, tag="xt")
                nc.sync.dma_start(out=xt, in_=x_tiles[i])
                for h in range(2):
                    pt = pxt.tile([P, 4, P], f32, tag="pxt")
                    for q in range(4):
                        j = h * 4 + q
                        nc.tensor.transpose(pt[:, q, :], xt[:, ts(j, P)], ident)
                    nc.scalar.activation(out=xT[:, i, h * 4:h * 4 + 4, :], in_=pt,
                                         func=AF.Copy)
                if i == 0:
                    nc.vector.tensor_copy(out=xacc, in_=xt)
                else:
                    nc.vector.tensor_add(out=xacc, in0=xacc, in1=xt)

            # ---- xsumT[c] = sum_t x[t, c] as 8 chunks of [128, 1] ----
            pxs = pacc.tile([P, NC], f32, tag="sm")
            for j in range(NC):
                nc.tensor.matmul(pxs[:, j:j + 1], lhsT=xacc[:, ts(j, P)],
                                 rhs=ones_col, start=True, stop=True)
            xsumT = small.tile([P, NC], f32, tag="xsumT")
            nc.vector.tensor_copy(out=xsumT, in_=pxs)

            # ---- ksumT = Wk.T @ xsum ; vsumT = Wv.T @ xsum  (both [128, 1]) ----
            pkv = pacc.tile([P, 2], f32, tag="sm")
            for j in range(NC):
                nc.tensor.matmul(pkv[:, 0:1], lhsT=wk_sb[:, j, :],
                                 rhs=xsumT[:, j:j + 1],
                                 start=(j == 0), stop=(j == NC - 1))
            for j in range(NC):
                nc.tensor.matmul(pkv[:, 1:2], lhsT=wv_sb[:, j, :],
                                 rhs=xsumT[:, j:j + 1],
                                 start=(j == 0), stop=(j == NC - 1))
            kv_sb = small.tile([P, 2], f32, tag="kv")
            nc.vector.tensor_copy(out=kv_sb, in_=pkv)

            # ---- wq = (Wq @ ksum) * alpha, as 8 chunks [128, 1] ----
            pwq = pacc.tile([P, NC], f32, tag="sm")
            for j in range(NC):
                nc.tensor.matmul(pwq[:, j:j + 1], lhsT=wqT[:, j, :],
                                 rhs=kv_sb[:, 0:1], start=True, stop=True)
            wq_vec = small.tile([P, NC], f32, tag="wq_vec")
            nc.vector.tensor_copy(out=wq_vec, in_=pwq)

            # ---- vsum broadcast to all partitions: vbc[t, d] = vsum[d] ----
            pvr = pacc.tile([1, P], f32, tag="sm")
            nc.tensor.transpose(pvr, kv_sb[:, 1:2], ident)
            vrow = small.tile([1, P], f32, tag="vrow")
            nc.vector.tensor_copy(out=vrow, in_=pvr)
            pvbc = pbig.tile([P, P], f32, tag="pvbc")
            nc.tensor.matmul(pvbc, lhsT=ones_row, rhs=vrow, start=True, stop=True)
            vbc = small.tile([P, P], f32, tag="vbc")
            nc.vector.tensor_copy(out=vbc, in_=pvbc)

            # ---- s[t] = x[t, :] . wq  -> psum [128, 16] (col i = token tile i) ----
            ps = pbig.tile([P, NT], f32, tag="ps")
            for i in range(NT):
                for j in range(NC):
                    nc.tensor.matmul(ps[:, i:i + 1], lhsT=xT[:, i, j, :],
                                     rhs=wq_vec[:, j:j + 1],
                                     start=(j == 0), stop=(j == NC - 1))

            # ---- softmax over all 2048 entries of s ----
            m1 = small.tile([P, 1], f32, tag="m1")
            nc.vector.reduce_max(out=m1, in_=ps, axis=mybir.AxisListType.X)
            pm = pacc.tile([1, P], f32, tag="sm")
            nc.tensor.transpose(pm, m1, ident)
            negm_s = small.tile([1, 1], f32, tag="negm_s")
            nc.vector.reduce_max(out=negm_s, in_=pm, axis=mybir.AxisListType.X,
                                 negate=True)
            pnm = pacc.tile([P, 1], f32, tag="sm")
            nc.tensor.matmul(pnm, lhsT=ones_row, rhs=negm_s, start=True, stop=True)
            negm = small.tile([P, 1], f32, tag="negm")
            nc.vector.tensor_copy(out=negm, in_=pnm)

            e_sb = small.tile([P, NT], f32, tag="e_sb")
            esum = small.tile([P, 1], f32, tag="esum")
            nc.scalar.activation(out=e_sb, in_=ps, func=AF.Exp, bias=negm,
                                 scale=1.0, accum_out=esum)

            pS = pacc.tile([1, 1], f32, tag="sm")
            nc.tensor.matmul(pS, lhsT=esum, rhs=ones_col, start=True, stop=True)
            r_s = small.tile([1, 1], f32, tag="r_s")
            nc.vector.reciprocal(out=r_s, in_=pS)
            pr = pacc.tile([P, 1], f32, tag="sm")
            nc.tensor.matmul(pr, lhsT=ones_row, rhs=r_s, start=True, stop=True)
            r_bc = small.tile([P, 1], f32, tag="r_bc")
            nc.vector.tensor_copy(out=r_bc, in_=pr)

            # ---- out[t, d] = e[t] * r * vsum[d]; DMA out per token tile ----
            out_sb = big.tile([P, NT, D], f32, tag="out_sb")
            import concourse.mybir as _mb
            for i in range(NT):
                nc.vector.tensor_scalar(out=out_sb[:, i, :], in0=vbc,
                                        scalar1=e_sb[:, i:i + 1], scalar2=r_bc,
                                        op0=_mb.AluOpType.mult,
                                        op1=_mb.AluOpType.mult)
                nc.sync.dma_start(out=out_view[:, i, :], in_=out_sb[:, i, :])

    nc.finalize()
    return nc


def _get_nc():
    if "nc" not in _NC_CACHE:
        _NC_CACHE["nc"] = build_bass()
    return _NC_CACHE["nc"]


def run(inputs, trace=False, **kwargs):
    """Run on 8 NeuronCores; returns (output [8, 2048, 128], BassKernelResults)."""
    from concourse.bass_utils import run_bass_kernel_spmd

    x = np.ascontiguousarray(np.asarray(inputs["x"], dtype=np.float32))
    Wq = np.ascontiguousarray(np.asarray(inputs["Wq"], dtype=np.float32))
    Wk = np.ascontiguousarray(np.asarray(inputs["Wk"], dtype=np.float32))
    Wv = np.ascontiguousarray(np.asarray(inputs["Wv"], dtype=np.float32))
    assert x.shape == (B, T, IN_C)

    nc = _get_nc()
    in_maps = [
        {"x": np.ascontiguousarray(x[i]), "Wq": Wq, "Wk": Wk, "Wv": Wv}
        for i in range(B)
    ]
    res = run_bass_kernel_spmd(nc, in_maps, core_ids=list(range(B)), trace=trace,
                               **kwargs)
    out = np.stack([np.asarray(res.results[i]["out"]) for i in range(B)], axis=0)
    return out.astype(np.float32), res


def kernel(**inputs) -> np.ndarray:
    out, _ = run(inputs, trace=False)
    return out



# revision 11
# speedup vs baseline: 1.3284x; 1.3284x over previous
"""Trainium2 Bass kernel for nn_AttentionHead_86715389706346.

Mathematical background
-----------------------
The reference module computes, per batch b:
    q = x @ Wq ; k = x @ Wk ; v = x @ Wv            (x: [T, C])
    attn = (q @ k.T) / sqrt(d)                       [T, T]
    attn = attn @ mask          (mask is all ones)
    p    = softmax(attn, axis=0)  (over the query axis)
    out  = p @ v

Because mask is the all-ones matrix, (attn @ mask)[q, t] = sum_k attn[q, k]
is independent of t.  The softmax over the *query* axis of a column-constant
matrix is also column-constant, so p[q, t] = softmax_q(s)[q] where

    s[q] = q[q, :] . ksum / sqrt(d),   ksum = sum_t k[t, :] = (sum_t x[t, :]) @ Wk

and the output collapses to a rank-1 outer product:

    out[q, d] = softmax(s)[q] * vsum[d],   vsum = (sum_t x[t, :]) @ Wv

This identity is exact (a reassociation of the same floating point sums).

Kernel structure (per core = per batch):
  phase 1 (DMA-bound): stream the 16 [128, 1024] x tiles into SBUF while the
    vector engine accumulates xacc = sum_i x_i and the tensor engine
    transposes Wq.
  chain: xsumT (8 matmuls vs ones) -> ksum (8 matmuls) -> w_row = ksum^T WqT
    (2 wide matmuls) -> broadcast w to all 128 partitions (2 matmuls).
  s-pass: s[t] = x[t, :].w computed with NO transpose of x: the vector
    engine uses fused tensor_tensor_reduce (multiply + free-axis reduce) on
    7 tiles while gpsimd multiplies and the scalar engine accum-reduces the
    other 9 tiles.
  softmax over all 2048 s values via tiny matmul reductions, then the
  rank-1 output out[t, :] = e[t]*r * vsum via per-partition scaled copies
  on the scalar + vector engines.

Distribution: data-parallel over batch; B == 8 == number of NeuronCores.
"""

import numpy as np

T = 2048
IN_C = 1024
D = 128
P = 128
NT = T // P      # 16 token tiles
NC = IN_C // P   # 8 channel chunks
B = 8
ALPHA = float(1.0 / np.sqrt(128.0))
N_DVE = 5        # s-pass tiles fully on the vector engine (mult + reduce)
USE_F32R = True  # use fp32r for the wide w_row/broadcast matmuls

_NC_CACHE = {}


def build_bass():
    import concourse.bass as bass
    import concourse.bacc as bacc
    import concourse.mybir as mybir
    import concourse.tile as tile
    from concourse.masks import make_identity
    from concourse.bass import ts

    f32 = mybir.dt.float32
    f32r = mybir.dt.float32r if USE_F32R else mybir.dt.float32
    AF = mybir.ActivationFunctionType
    OP = mybir.AluOpType

    nc = bacc.Bacc()
    x_ext = nc.declare_dram_parameter("x", [T, IN_C], f32, isOutput=False)
    wq_ext = nc.declare_dram_parameter("Wq", [IN_C, D], f32, isOutput=False)
    wk_ext = nc.declare_dram_parameter("Wk", [IN_C, D], f32, isOutput=False)
    wv_ext = nc.declare_dram_parameter("Wv", [IN_C, D], f32, isOutput=False)
    out_ext = nc.declare_dram_parameter("out", [T, D], f32, isOutput=True)

    x_tiles = x_ext.rearrange("(i p) c -> i p c", p=P)        # [16, 128, 1024]
    out_view = out_ext.rearrange("(i p) d -> p i d", p=P)     # [128, 16, 128]

    with tile.TileContext(nc) as tc:
        with (
            tc.tile_pool(name="const", bufs=1) as cpool,
            tc.tile_pool(name="xbuf", bufs=1) as xbuf,
            tc.tile_pool(name="wbuf", bufs=1) as wbuf,
            tc.tile_pool(name="work", bufs=1) as work,
            tc.tile_pool(name="scr", bufs=2) as scr,
            tc.tile_pool(name="pacc", bufs=2, space="PSUM") as pacc,
            tc.tile_pool(name="pt", bufs=2, space="PSUM") as ptp,
            tc.tile_pool(name="pv", bufs=1, space="PSUM") as pvp,
            tc.tile_pool(name="pper", bufs=1, space="PSUM") as pper,
        ):
            # ---- constants ----
            ident = cpool.tile([P, P], f32)
            make_identity(nc, ident)
            ones_col = cpool.tile([P, 1], f32)
            nc.vector.memset(ones_col, 1.0)
            ones_row = cpool.tile([1, P], f32)
            nc.vector.memset(ones_row, 1.0)
            ones_row_r = cpool.tile([1, P], f32r)
            nc.vector.tensor_copy(out=ones_row_r, in_=ones_row)

            # ---- weights: [c_in, chunk, d] ----
            wq_sb = wbuf.tile([P, NC, D], f32)
            nc.sync.dma_start(out=wq_sb, in_=wq_ext.rearrange("(j c) d -> c j d", c=P))
            wk_sb = wbuf.tile([P, NC, D], f32)
            nc.sync.dma_start(out=wk_sb, in_=wk_ext.rearrange("(j c) d -> c j d", c=P))
            wv_sb = wbuf.tile([P, NC, D], f32)
            nc.sync.dma_start(out=wv_sb, in_=wv_ext.rearrange("(j c) d -> c j d", c=P))

            # WqT: [d, chunk, c] (for w_row = ksum^T @ WqT), rounded to fp32r
            wqT = wbuf.tile([P, NC, P], f32r)
            for j in range(NC):
                pt = ptp.tile([P, P], f32, tag="pt")
                nc.tensor.transpose(pt, wq_sb[:, j, :], ident)
                nc.scalar.activation(out=wqT[:, j, :], in_=pt, func=AF.Copy)

            # Preload the exp table early (off critical path).
            dummy = work.tile([P, 1], f32, tag="dummy")
            nc.scalar.activation(out=dummy, in_=ones_col, func=AF.Exp)

            # ---- phase 1: stream x, accumulate xacc on the vector engine ----
            x_all = xbuf.tile([P, NT, IN_C], f32, tag="x_all")
            xacc = work.tile([P, IN_C], f32, tag="xacc")
            for i in range(NT):
                nc.sync.dma_start(out=x_all[:, i, :], in_=x_tiles[i])
                if i == 0:
                    nc.vector.tensor_copy(out=xacc, in_=x_all[:, 0, :])
                else:
                    nc.vector.tensor_add(out=xacc, in0=xacc, in1=x_all[:, i, :])

            # ---- chain step 1: xsumT[c] = sum_t' xacc[t', c]  -> [128, 8] ----
            pxs = pacc.tile([P, NC], f32, tag="sm")
            for j in range(NC):
                nc.tensor.matmul(pxs[:, j:j + 1], lhsT=xacc[:, ts(j, P)],
                                 rhs=ones_col, start=True, stop=True)
            xsT_sb = work.tile([P, NC], f32, tag="xsT")
            nc.vector.tensor_copy(out=xsT_sb, in_=pxs)

            # ---- chain step 2: ksum = Wk^T @ xsum  -> [128, 1] ----
            pk = pacc.tile([P, 1], f32, tag="sm")
            for j in range(NC):
                nc.tensor.matmul(pk, lhsT=wk_sb[:, j, :], rhs=xsT_sb[:, j:j + 1],
                                 start=(j == 0), stop=(j == NC - 1))
            ksum_sb = work.tile([P, 1], f32r, tag="ksum")
            nc.vector.tensor_copy(out=ksum_sb, in_=pk)

            # ---- chain step 3: w_row[1, 1024] = ksum^T @ WqT (fp32r, wide) ----
            pw0 = pacc.tile([1, 512], f32, tag="sm")
            pw1 = pacc.tile([1, 512], f32, tag="sm")
            nc.tensor.matmul(pw0, lhsT=ksum_sb, rhs=wqT[:, 0:4, :],
                             start=True, stop=True)
            nc.tensor.matmul(pw1, lhsT=ksum_sb, rhs=wqT[:, 4:8, :],
                             start=True, stop=True)
            w_row = work.tile([1, IN_C], f32r, tag="w_row")
            nc.scalar.activation(out=w_row[:, 0:512], in_=pw0, func=AF.Copy)
            nc.vector.tensor_copy(out=w_row[:, 512:1024], in_=pw1)

            # ---- chain step 4: broadcast w to all partitions -> SBUF ----
            pwb0 = pper.tile([P, 512], f32, tag="pwb0")
            pwb1 = pper.tile([P, 512], f32, tag="pwb1")
            nc.tensor.matmul(pwb0, lhsT=ones_row_r, rhs=w_row[:, 0:512],
                             start=True, stop=True)
            nc.tensor.matmul(pwb1, lhsT=ones_row_r, rhs=w_row[:, 512:1024],
                             start=True, stop=True)
            w_bc = work.tile([P, IN_C], f32, tag="w_bc")
            nc.scalar.activation(out=w_bc[:, 0:512], in_=pwb0, func=AF.Copy)
            nc.vector.tensor_copy(out=w_bc[:, 512:1024], in_=pwb1)

            # ---- vsum (off critical path; tensor engine is idle in s-pass) ----
            pv = pvp.tile([P, 1], f32, tag="pv")
            for j in range(NC):
                nc.tensor.matmul(pv, lhsT=wv_sb[:, j, :], rhs=xsT_sb[:, j:j + 1],
                                 start=(j == 0), stop=(j == NC - 1))
            vsum_sb = work.tile([P, 1], f32, tag="vsum")
            nc.vector.tensor_copy(out=vsum_sb, in_=pv)
            pvr = ptp.tile([1, P], f32, tag="pt")
            nc.tensor.transpose(pvr, vsum_sb, ident)
            vrow_sb = work.tile([1, P], f32, tag="vrow")
            nc.scalar.activation(out=vrow_sb, in_=pvr, func=AF.Copy)
            pvbc = pper.tile([P, P], f32, tag="pvbc")
            nc.tensor.matmul(pvbc, lhsT=ones_row, rhs=vrow_sb, start=True,
                             stop=True)

            # ---- s-pass: s[t] = x[t, :] . w   (no transpose of x) ----
            # gpsimd multiplies 11 tiles, the scalar engine reduces them via
            # Identity+accum_out; the vector engine does 5 tiles end-to-end.
            s_sb = work.tile([P, NT], f32, tag="s_sb")
            trash = work.tile([P, IN_C], f32, tag="trash")
            dve_tiles = list(range(N_DVE))
            pool_tiles = list(range(N_DVE, NT))
            order = []
            for a, b in zip(dve_tiles, pool_tiles):
                order += [(b, "pool"), (a, "dve")]
            order += [(i, "dve") for i in dve_tiles[len(pool_tiles):]]
            order += [(i, "pool") for i in pool_tiles[len(dve_tiles):]]
            for i, eng in order:
                if eng == "dve":
                    zd = scr.tile([P, IN_C], f32, tag="zd")
                    nc.vector.tensor_tensor(out=zd, in0=x_all[:, i, :],
                                            in1=w_bc, op=OP.mult)
                    nc.vector.tensor_reduce(out=s_sb[:, i:i + 1], in_=zd,
                                            axis=mybir.AxisListType.X,
                                            op=OP.add)
                else:
                    zp = scr.tile([P, IN_C], f32, tag="zp")
                    nc.gpsimd.tensor_mul(out=zp, in0=x_all[:, i, :], in1=w_bc)
                    nc.scalar.activation(out=trash, in_=zp, func=AF.Identity,
                                         accum_out=s_sb[:, i:i + 1])

            # ---- softmax over all 2048 entries of s ----
            m1 = work.tile([P, 1], f32, tag="m1")
            nc.vector.reduce_max(out=m1, in_=s_sb, axis=mybir.AxisListType.X)
            pm = pacc.tile([1, P], f32, tag="sm")
            nc.tensor.transpose(pm, m1, ident)
            negm_s = work.tile([1, 1], f32, tag="negm_s")
            nc.vector.reduce_max(out=negm_s, in_=pm, axis=mybir.AxisListType.X,
                                 negate=True)
            pnm = pacc.tile([P, 1], f32, tag="sm")
            nc.tensor.matmul(pnm, lhsT=ones_row, rhs=negm_s, start=True,
                             stop=True)
            negam = work.tile([P, 1], f32, tag="negam")
            nc.vector.tensor_scalar(out=negam, in0=pnm, scalar1=ALPHA,
                                    scalar2=None, op0=OP.mult)

            e_sb = work.tile([P, NT], f32, tag="e_sb")
            esum = work.tile([P, 1], f32, tag="esum")
            nc.scalar.activation(out=e_sb, in_=s_sb, func=AF.Exp, bias=negam,
                                 scale=ALPHA, accum_out=esum)

            pS = pacc.tile([1, 1], f32, tag="sm")
            nc.tensor.matmul(pS, lhsT=esum, rhs=ones_col, start=True, stop=True)
            r_s = work.tile([1, 1], f32, tag="r_s")
            nc.vector.reciprocal(out=r_s, in_=pS)
            pr = pacc.tile([P, 1], f32, tag="sm")
            nc.tensor.matmul(pr, lhsT=ones_row, rhs=r_s, start=True, stop=True)
            r_bc = work.tile([P, 1], f32, tag="r_bc")
            nc.vector.tensor_copy(out=r_bc, in_=pr)
            er_sb = work.tile([P, NT], f32, tag="er_sb")
            nc.vector.tensor_scalar(out=er_sb, in0=e_sb, scalar1=r_bc,
                                    scalar2=None, op0=OP.mult)

            # ---- out[t, d] = er[t] * vsum[d]; split scalar/vector engines ----
            out_sb = xbuf.tile([P, NT, D], f32, tag="out_sb")
            for i in range(NT):
                if i % 2 == 0:
                    nc.scalar.activation(out=out_sb[:, i, :], in_=pvbc,
                                         func=AF.Copy, scale=er_sb[:, i:i + 1])
                else:
                    nc.vector.tensor_scalar(out=out_sb[:, i, :], in0=pvbc,
                                            scalar1=er_sb[:, i:i + 1],
                                            scalar2=None, op0=OP.mult)
                nc.sync.dma_start(out=out_view[:, i, :], in_=out_sb[:, i, :])

    nc.finalize()
    return nc


def _get_nc():
    if "nc" not in _NC_CACHE:
        _NC_CACHE["nc"] = build_bass()
    return _NC_CACHE["nc"]


def run(inputs, trace=False, **kwargs):
    """Run on 8 NeuronCores; returns (output [8, 2048, 128], BassKernelResults)."""
    from concourse.bass_utils import run_bass_kernel_spmd

    x = np.ascontiguousarray(np.asarray(inputs["x"], dtype=np.float32))
    Wq = np.ascontiguousarray(np.asarray(inputs["Wq"], dtype=np.float32))
    Wk = np.ascontiguousarray(np.asarray(inputs["Wk"], dtype=np.float32))
    Wv = np.ascontiguousarray(np.asarray(inputs["Wv"], dtype=np.float32))
    assert x.shape == (B, T, IN_C)

    nc = _get_nc()
    in_maps = [
        {"x": np.ascontiguousarray(x[i]), "Wq": Wq, "Wk": Wk, "Wv": Wv}
        for i in range(B)
    ]
    res = run_bass_kernel_spmd(nc, in_maps, core_ids=list(range(B)), trace=trace,
                               **kwargs)
    out = np.stack([np.asarray(res.results[i]["out"]) for i in range(B)], axis=0)
    return out.astype(np.float32), res


def kernel(**inputs) -> np.ndarray:
    out, _ = run(inputs, trace=False)
    return out


# revision 16
# speedup vs baseline: 1.5684x; 1.1806x over previous
"""Trainium2 Bass kernel for nn_AttentionHead_86715389706346.

Mathematical background
-----------------------
The reference module computes, per batch b:
    q = x @ Wq ; k = x @ Wk ; v = x @ Wv            (x: [T, C])
    attn = (q @ k.T) / sqrt(d)                       [T, T]
    attn = attn @ mask          (mask is all ones)
    p    = softmax(attn, axis=0)  (over the query axis)
    out  = p @ v

Because mask is the all-ones matrix, (attn @ mask)[q, t] = sum_k attn[q, k]
is independent of t.  The softmax over the *query* axis of a column-constant
matrix is also column-constant, so p[q, t] = softmax_q(s)[q] where

    s[q] = q[q, :] . ksum / sqrt(d),   ksum = sum_t k[t, :] = (sum_t x[t, :]) @ Wk

and the output collapses to a rank-1 outer product:

    out[q, d] = softmax(s)[q] * vsum[d],   vsum = (sum_t x[t, :]) @ Wv

This identity is exact (a reassociation of the same floating point sums).

Kernel structure (per core = per batch):
  phase 1 (DMA + vector): stream 16 [128, 1024] x tiles (x DMAs issued
    first; weights use a 4 KiB-per-descriptor permuted layout so they do
    not clog the DMA queues); the vector engine accumulates xacc.
  chain (fp32r single-pass matmuls): xsumT (8) -> ksum (8) -> w_row =
    ksum^T WqT (2 wide) -> broadcast to all partitions (2 wide, with the
    channel permutation undone by a strided rhs access pattern).
  s-pass: s[t] = x[t, :].w via 16 fused affine_mul_reduce ops on the
    vector engine (one pass over x, no transpose, no gpsimd -- gpsimd
    software ops contend with the vector engine for SBUF ports).
  softmax over the 2048 s values, then the rank-1 output via per-partition
  scaled copies split across the scalar + vector engines.

Weights are loaded permuted: w2[p, j, d] = W[8p+j, d] so each SBUF
partition line is one 4 KiB contiguous HBM read.  All c-contractions are
permutation invariant as long as both operands use the same order; the
broadcast matmul restores natural channel order via its rhs AP.

Distribution: data-parallel over batch; B == 8 == number of NeuronCores.
"""

import numpy as np

T = 2048
IN_C = 1024
D = 128
P = 128
NT = T // P      # 16 token tiles
NC = IN_C // P   # 8 channel chunks
B = 8
ALPHA = float(1.0 / np.sqrt(128.0))

_NC_CACHE = {}


def build_bass():
    import concourse.bass as bass
    import concourse.bacc as bacc
    import concourse.mybir as mybir
    import concourse.tile as tile
    from concourse.masks import make_identity

    f32 = mybir.dt.float32
    f32r = mybir.dt.float32r
    AF = mybir.ActivationFunctionType
    OP = mybir.AluOpType

    nc = bacc.Bacc()
    x_ext = nc.declare_dram_parameter("x", [T, IN_C], f32, isOutput=False)
    wq_ext = nc.declare_dram_parameter("Wq", [IN_C, D], f32, isOutput=False)
    wk_ext = nc.declare_dram_parameter("Wk", [IN_C, D], f32, isOutput=False)
    wv_ext = nc.declare_dram_parameter("Wv", [IN_C, D], f32, isOutput=False)
    out_ext = nc.declare_dram_parameter("out", [T, D], f32, isOutput=True)

    x_tiles = x_ext.rearrange("(i p) c -> i p c", p=P)        # [16, 128, 1024]
    out_view = out_ext.rearrange("(i p) d -> p i d", p=P)     # [128, 16, 128]

    with tile.TileContext(nc) as tc:
        with (
            tc.tile_pool(name="const", bufs=1) as cpool,
            tc.tile_pool(name="xbuf", bufs=1) as xbuf,
            tc.tile_pool(name="wbuf", bufs=1) as wbuf,
            tc.tile_pool(name="work", bufs=1) as work,
            tc.tile_pool(name="scr", bufs=2) as scr,
            tc.tile_pool(name="pacc", bufs=2, space="PSUM") as pacc,
            tc.tile_pool(name="pt", bufs=2, space="PSUM") as ptp,
            tc.tile_pool(name="pv", bufs=1, space="PSUM") as pvp,
            tc.tile_pool(name="pper", bufs=1, space="PSUM") as pper,
        ):
            # ---- x DMAs first: they own the queue heads ----
            x_all = xbuf.tile([P, NT, IN_C], f32, tag="x_all")
            for i in range(NT):
                nc.sync.dma_start(out=x_all[:, i, :], in_=x_tiles[i])

            # ---- weights, permuted [p, j, d] = W[8p+j, d]: 4KiB descriptors
            wq2 = wbuf.tile([P, NC, D], f32)
            nc.sync.dma_start(out=wq2, in_=wq_ext.rearrange("(c j) d -> c j d", j=NC))
            wk2 = wbuf.tile([P, NC, D], f32)
            nc.sync.dma_start(out=wk2, in_=wk_ext.rearrange("(c j) d -> c j d", j=NC))
            wv2 = wbuf.tile([P, NC, D], f32)
            nc.sync.dma_start(out=wv2, in_=wv_ext.rearrange("(c j) d -> c j d", j=NC))

            # ---- constants ----
            ident = cpool.tile([P, P], f32)
            make_identity(nc, ident)
            ones_col = cpool.tile([P, 1], f32)
            nc.vector.memset(ones_col, 1.0)
            ones_row = cpool.tile([1, P], f32)
            nc.vector.memset(ones_row, 1.0)
            ones_row_r = cpool.tile([1, P], f32r)
            nc.vector.tensor_copy(out=ones_row_r, in_=ones_row)
            ones_col2 = cpool.tile([P, 2], f32)
            nc.vector.memset(ones_col2, 1.0)
            ones_col2_r = cpool.tile([P, 2], f32r)
            nc.vector.tensor_copy(out=ones_col2_r, in_=ones_col2)

            # Preload the exp table early (off critical path).
            dummy = work.tile([P, 1], f32, tag="dummy")
            nc.scalar.activation(out=dummy, in_=ones_col, func=AF.Exp)

            # fp32r copies of Wk/Wv (scalar engine, hidden under phase 1)
            wk2r = wbuf.tile([P, NC, D], f32r)
            nc.scalar.activation(out=wk2r, in_=wk2, func=AF.Copy)
            wv2r = wbuf.tile([P, NC, D], f32r)
            nc.scalar.activation(out=wv2r, in_=wv2, func=AF.Copy)

            # WqT2[d, j, p] = Wq[8p+j, d], rounded to fp32r
            wqT2 = wbuf.tile([P, NC, P], f32r)
            for j in range(NC):
                pt = ptp.tile([P, P], f32, tag="pt")
                nc.tensor.transpose(pt, wq2[:, j, :], ident)
                nc.scalar.activation(out=wqT2[:, j, :], in_=pt, func=AF.Copy)

            # ---- phase 1: accumulate xacc on the vector engine ----
            xacc = work.tile([P, IN_C], f32, tag="xacc")
            for i in range(NT):
                if i == 0:
                    nc.vector.tensor_copy(out=xacc, in_=x_all[:, 0, :])
                else:
                    nc.vector.tensor_add(out=xacc, in0=xacc, in1=x_all[:, i, :])
            xacc_r = work.tile([P, IN_C], f32r, tag="xacc_r")
            nc.vector.tensor_copy(out=xacc_r, in_=xacc)

            # ---- chain 1: xsT2[p, j] = xsum[8p+j]  (8 fp32r matmuls) ----
            xacc_rv = xacc_r.rearrange("t (c j) -> t j c", j=NC)
            pxs = pacc.tile([P, 2 * NC], f32, tag="sm")
            for j in range(NC):
                nc.tensor.matmul(pxs[:, 2 * j:2 * j + 2], lhsT=xacc_rv[:, j, :],
                                 rhs=ones_col2_r, start=True, stop=True)
            xsT_sb = work.tile([P, 2 * NC], f32r, tag="xsT")
            nc.vector.tensor_copy(out=xsT_sb, in_=pxs)

            # ---- chain 2: ksum[d] = sum_c xsum[c] Wk[c, d]  (8 matmuls) ----
            pk = pacc.tile([P, 2], f32, tag="sm")
            for j in range(NC):
                nc.tensor.matmul(pk, lhsT=wk2r[:, j, :],
                                 rhs=xsT_sb[:, 2 * j:2 * j + 2],
                                 start=(j == 0), stop=(j == NC - 1))
            ksum_sb = work.tile([P, 1], f32r, tag="ksum")
            nc.vector.tensor_copy(out=ksum_sb, in_=pk[:, 0:1])

            # ---- chain 3: w_row2[1, (j p)] = ksum^T @ WqT  (2 wide fp32r) ----
            pw0 = pacc.tile([1, 512], f32, tag="sm")
            pw1 = pacc.tile([1, 512], f32, tag="sm")
            nc.tensor.matmul(pw0, lhsT=ksum_sb, rhs=wqT2[:, 0:4, :],
                             start=True, stop=True)
            nc.tensor.matmul(pw1, lhsT=ksum_sb, rhs=wqT2[:, 4:8, :],
                             start=True, stop=True)
            w_row = work.tile([1, IN_C], f32r, tag="w_row")
            nc.scalar.activation(out=w_row[:, 0:512], in_=pw0, func=AF.Copy)
            nc.vector.tensor_copy(out=w_row[:, 512:1024], in_=pw1)

            # ---- chain 4: broadcast + un-permute w -> natural order ----
            # natural c = 8p+j: stream rhs in (p, j) order
            w_rv = w_row.rearrange("o (j p) -> o p j", j=NC)   # [1, 128, 8]
            pwb0 = pper.tile([P, 512], f32, tag="pwb0")
            pwb1 = pper.tile([P, 512], f32, tag="pwb1")
            nc.tensor.matmul(pwb0, lhsT=ones_row_r, rhs=w_rv[:, 0:64, :],
                             start=True, stop=True)
            nc.tensor.matmul(pwb1, lhsT=ones_row_r, rhs=w_rv[:, 64:128, :],
                             start=True, stop=True)
            w_bc = work.tile([P, IN_C], f32, tag="w_bc")
            nc.scalar.activation(out=w_bc[:, 0:512], in_=pwb0, func=AF.Copy)
            nc.vector.tensor_copy(out=w_bc[:, 512:1024], in_=pwb1)

            # ---- vsum (tensor engine, runs during the s-pass) ----
            pv = pvp.tile([P, 2], f32, tag="pv")
            for j in range(NC):
                nc.tensor.matmul(pv, lhsT=wv2r[:, j, :],
                                 rhs=xsT_sb[:, 2 * j:2 * j + 2],
                                 start=(j == 0), stop=(j == NC - 1))
            vsum_sb = work.tile([P, 1], f32, tag="vsum")
            nc.scalar.activation(out=vsum_sb, in_=pv[:, 0:1], func=AF.Copy)
            pvr = ptp.tile([1, P], f32, tag="pt")
            nc.tensor.transpose(pvr, vsum_sb, ident)
            vrow_sb = work.tile([1, P], f32, tag="vrow")
            nc.scalar.activation(out=vrow_sb, in_=pvr, func=AF.Copy)
            pvbc = pper.tile([P, P], f32, tag="pvbc")
            nc.tensor.matmul(pvbc, lhsT=ones_row, rhs=vrow_sb, start=True,
                             stop=True)

            # ---- s-pass: 16 fused multiply+reduce on the vector engine ----
            s_sb = work.tile([P, NT], f32, tag="s_sb")
            for i in range(NT):
                zd = scr.tile([P, IN_C], f32, tag="zd")
                nc.vector.affine_mul_reduce(out=zd, accum_out=s_sb[:, i:i + 1],
                                            in0=x_all[:, i, :], in1=w_bc,
                                            scale=1.0, bias=0.0)

            # ---- softmax over all 2048 entries of s ----
            m1 = work.tile([P, 1], f32, tag="m1")
            nc.vector.reduce_max(out=m1, in_=s_sb, axis=mybir.AxisListType.X)
            pm = pacc.tile([1, P], f32, tag="sm")
            nc.tensor.transpose(pm, m1, ident)
            negm_s = work.tile([1, 1], f32, tag="negm_s")
            nc.vector.reduce_max(out=negm_s, in_=pm, axis=mybir.AxisListType.X,
                                 negate=True)
            pnm = pacc.tile([P, 1], f32, tag="sm")
            nc.tensor.matmul(pnm, lhsT=ones_row, rhs=negm_s, start=True,
                             stop=True)
            negam = work.tile([P, 1], f32, tag="negam")
            nc.vector.tensor_scalar(out=negam, in0=pnm, scalar1=ALPHA,
                                    scalar2=None, op0=OP.mult)

            e_sb = work.tile([P, NT], f32, tag="e_sb")
            esum = work.tile([P, 1], f32, tag="esum")
            nc.scalar.activation(out=e_sb, in_=s_sb, func=AF.Exp, bias=negam,
                                 scale=ALPHA, accum_out=esum)

            pS = pacc.tile([1, 1], f32, tag="sm")
            nc.tensor.matmul(pS, lhsT=esum, rhs=ones_col, start=True, stop=True)
            r_s = work.tile([1, 1], f32, tag="r_s")
            nc.vector.reciprocal(out=r_s, in_=pS)
            pr = pacc.tile([P, 1], f32, tag="sm")
            nc.tensor.matmul(pr, lhsT=ones_row, rhs=r_s, start=True,
                             stop=True)
            r_bc = work.tile([P, 1], f32, tag="r_bc")
            nc.vector.tensor_copy(out=r_bc, in_=pr)
            er_sb = work.tile([P, NT], f32, tag="er_sb")
            nc.vector.tensor_scalar(out=er_sb, in0=e_sb, scalar1=r_bc,
                                    scalar2=None, op0=OP.mult)

            # ---- out[t, d] = er[t] * vsum[d]; split scalar/vector engines ----
            out_sb = xbuf.tile([P, NT, D], f32, tag="out_sb")
            for i in range(NT):
                if i % 2 == 0:
                    nc.scalar.activation(out=out_sb[:, i, :], in_=pvbc,
                                         func=AF.Copy, scale=er_sb[:, i:i + 1])
                else:
                    nc.vector.tensor_scalar(out=out_sb[:, i, :], in0=pvbc,
                                            scalar1=er_sb[:, i:i + 1],
                                            scalar2=None, op0=OP.mult)
                nc.sync.dma_start(out=out_view[:, i, :], in_=out_sb[:, i, :])

    nc.finalize()
    return nc


def _get_nc():
    if "nc" not in _NC_CACHE:
        _NC_CACHE["nc"] = build_bass()
    return _NC_CACHE["nc"]


def run(inputs, trace=False, **kwargs):
    """Run on 8 NeuronCores; returns (output [8, 2048, 128], BassKernelResults)."""
    from concourse.bass_utils import run_bass_kernel_spmd

    x = np.ascontiguousarray(np.asarray(inputs["x"], dtype=np.float32))
    Wq = np.ascontiguousarray(np.asarray(inputs["Wq"], dtype=np.float32))
    Wk = np.ascontiguousarray(np.asarray(inputs["Wk"], dtype=np.float32))
    Wv = np.ascontiguousarray(np.asarray(inputs["Wv"], dtype=np.float32))
    assert x.shape == (B, T, IN_C)

    nc = _get_nc()
    in_maps = [
        {"x": np.ascontiguousarray(x[i]), "Wq": Wq, "Wk": Wk, "Wv": Wv}
        for i in range(B)
    ]
    res = run_bass_kernel_spmd(nc, in_maps, core_ids=list(range(B)), trace=trace,
                               **kwargs)
    out = np.stack([np.asarray(res.results[i]["out"]) for i in range(B)], axis=0)
    return out.astype(np.float32), res


def kernel(**inputs) -> np.ndarray:
    out, _ = run(inputs, trace=False)
    return out
